# revision 55
# baseline (speedup 1.0000x reference)
"""Trainium2 Bass kernel for nn_MultiHeadAttention (B=2, S=2048, H=1024, 16 heads).

Sharding: 8 cores = 2 (batch) x 4 (head-groups of 4 heads). Each core computes
QKV projections for its 256-dim head slice, attention for its 4 heads, and a
partial output projection. Host sums the 4 head-group partials per batch and
adds the output bias.

Pipeline: the attention g-loop (scores -> exp -> attn@V) is paced by the ACT
engine (exp is irreducible: 131072 columns/core). Independent matmul work --
Q projection for the next q-block, output projection for the previous one,
deferred K-projection chunks -- is interleaved into the PE stream as fillers
so the PE never idles waiting for exp and ACT never idles waiting for scores
PSUM buffers.

All matmuls are emitted as full 128x128 tiles (the PE streams full-tile fp16
at 2 cols/cycle vs 1 for partial tiles): the scores stationary comes from
zero-padded even/odd K tiles and V' is zero-padded to 128 columns. V'
([s, d]-layout V with a fused ones column for softmax row sums) is built
directly by the V projection using x-chunks as the stationary operand, with
the V bias folded in as a K=1 ones-row matmul. Softmax normalization batches
one reciprocal per head-pair on partitions {0, 32} (gathered by tiny SBUF
DMAs) and broadcasts 1/sum with an fp16 K=1 matmul.
"""

import sys

if "/opt/trn_rl_repo" not in sys.path:
    sys.path.insert(0, "/opt/trn_rl_repo")

import numpy as np

HIDDEN, HEADS, D_K, B, S = 1024, 16, 64, 2, 2048
G = 4              # head groups (tensor-parallel dim)
HPG = HEADS // G   # heads per group
DSL = HPG * D_K    # 256: d-slice per core
P = 128
QB = 512           # q-block size for attention tiling
N_QB = S // QB     # 4
KC = S // P        # 16 k-chunks
NG = KC // 2       # 8 two-chunk groups
CC = HIDDEN // P   # 8 contraction chunks for projections
SCALE = 1.0 / np.sqrt(np.float32(D_K))
D = 3              # attn@V trails the scores/exp stream by D 2-chunk groups


def _build_nc():
    import concourse.mybir as mybir
    import concourse.tile as tile
    from concourse.bacc import Bacc

    dt = mybir.dt
    f32 = dt.float32
    f16 = dt.float16

    nc = Bacc(None)

    qT_d = nc.dram_tensor("qT", [HIDDEN, S], f16, kind="ExternalInput")
    kT_d = nc.dram_tensor("kT", [HIDDEN, S], f16, kind="ExternalInput")
    vT_d = nc.dram_tensor("vT", [HIDDEN, S], f16, kind="ExternalInput")
    wqT_d = nc.dram_tensor("wqT", [HIDDEN, DSL], f16, kind="ExternalInput")
    wkT_d = nc.dram_tensor("wkT", [HIDDEN, DSL], f16, kind="ExternalInput")
    wvT_d = nc.dram_tensor("wvT", [HIDDEN, DSL], f16, kind="ExternalInput")
    woT_d = nc.dram_tensor("woT", [DSL, HIDDEN], f16, kind="ExternalInput")
    bq_d = nc.dram_tensor("bq", [DSL], f32, kind="ExternalInput")
    bk_d = nc.dram_tensor("bk", [DSL], f32, kind="ExternalInput")
    bv_d = nc.dram_tensor("bv", [DSL], f16, kind="ExternalInput")
    y_d = nc.dram_tensor("y", [S, HIDDEN], f32, kind="ExternalOutput")
    y_r = y_d.rearrange("(sc p) e -> p sc e", p=P)

    with tile.TileContext(nc) as tc:
        with (
            tc.tile_pool(name="weights", bufs=1) as wpool,
            tc.tile_pool(name="xq_pool", bufs=1) as xqpool,
            tc.tile_pool(name="kqT", bufs=1) as kqpool,
            tc.tile_pool(name="vprime", bufs=1) as vpool,
            tc.tile_pool(name="xT_out", bufs=1) as xtpool,
            tc.tile_pool(name="expc_p", bufs=1) as epool,
            tc.tile_pool(name="small", bufs=1) as small,
        ):
            ones16 = small.tile([P, P], f16, tag="ones")
            nc.vector.memset(ones16[:], 1.0)

            # persistent activations / outputs. K lives in two zero-padded
            # tiles (even/odd head at partitions 0-63/64-127, other half 0)
            # so the scores matmul is a full 128x128 tile: the PE streams
            # full-tile fp16 matmuls at 2 cols/cycle vs 1 for partial tiles.
            KTZ = [
                kqpool.tile([P, DSL // P, S], f16, tag=f"KTZ{par}",
                            name=f"KTZ{par}")
                for par in range(2)
            ]
            nc.vector.memset(KTZ[0][:], 0.0)
            nc.vector.memset(KTZ[1][:], 0.0)
            QT = kqpool.tile([P, DSL // P, S], f16, tag="QT", name="QT")
            # V' per head: [s, d] layout, ones column at d=D_K for softmax
            # sums, zero-padded to 128 columns for the full-tile fast path.
            vpc = vpool.tile([P, KC, HPG, P], f16, tag="vpc", name="vpc")
            nc.vector.memset(vpc[:], 0.0)
            nc.vector.memset(vpc[:, :, :, D_K : D_K + 1], 1.0)
            XT = xtpool.tile([P, DSL // P, S], f16, tag="XT", name="XT")
            # exp scores, combined for both heads of the active pair
            expc = epool.tile([P, KC, 2, QB], f16, tag="expc", name="expc")

            # weights needed across the whole kernel
            wq_t = wpool.tile([P, CC, DSL], f16, tag="wq", name="wq_t")
            wk_t = wpool.tile([P, CC, DSL], f16, tag="wk", name="wk_t")
            woT_sb = wpool.tile([P, DSL // P, HIDDEN], f16, tag="wo", name="woT_sb")
            bq_t = wpool.tile([P, DSL // P], f32, tag="bq", name="bq_t")
            bk_t = wpool.tile([P, DSL // P], f32, tag="bk", name="bk_t")
            bv16 = wpool.tile([1, DSL], f16, tag="bv", name="bv16")
            xq_ts = [
                xqpool.tile([P, CC // 2, S], f16, tag=f"xq{hf}", name=f"xq{hf}")
                for hf in range(2)
            ]
            xk_ts = [
                xqpool.tile([P, CC // 2, S], f16, tag=f"xk{hf}", name=f"xk{hf}")
                for hf in range(2)
            ]

            def emit_proj_chunk(out_t, w_t, b_t, xts, psum_pool, mc, ns,
                                bias_on_act=False, split_halves=False):
                # one [128, 512] column block of a [d, s]-layout projection;
                # split_halves routes the two 64-partition halves into the
                # zero-padded even/odd K tiles.
                ps = psum_pool.tile([P, QB], f32, tag="rby", name=f"pp{mc}_{ns}")
                for cc in range(CC):
                    nc.tensor.matmul(
                        ps[:],
                        w_t[:, cc, mc * P : (mc + 1) * P],
                        xts[cc // 4][:, cc % 4, ns * QB : (ns + 1) * QB],
                        start=(cc == 0),
                        stop=(cc == CC - 1),
                    )
                if split_halves:
                    dsts = [
                        (out_t[par][slice(par * D_K, par * D_K + D_K), mc,
                                    ns * QB : (ns + 1) * QB],
                         slice(par * D_K, par * D_K + D_K))
                        for par in range(2)
                    ]
                else:
                    dsts = [
                        (out_t[:, mc, ns * QB : (ns + 1) * QB], slice(0, P))
                    ]
                for dst, rows in dsts:
                    if bias_on_act:
                        # ACT is idle in the head phase; bias is per-partition
                        nc.scalar.activation(
                            dst, ps[rows, :],
                            mybir.ActivationFunctionType.Identity,
                            bias=b_t[rows, mc : mc + 1],
                        )
                    else:
                        nc.vector.tensor_scalar_add(
                            dst, ps[rows, :], b_t[rows, mc : mc + 1]
                        )

            # ---- head phase: k proj (ns 0-1), v' direct proj, q proj (qb0) ----
            with (
                tc.tile_pool(name="head_x", bufs=1) as hx,
                tc.tile_pool(name="head_w", bufs=1) as hwp,
                tc.tile_pool(name="proj_ps", bufs=4, space="PSUM") as proj_ps,
                tc.tile_pool(name="v_ps", bufs=2, space="PSUM") as v_ps,
            ):
                # DMAs in consumption order; the first k-proj block waits
                # only on wk's first half + the ns0 column stripes, and
                # vdirect's first chunks interleave right behind.
                wkr = wkT_d.rearrange("(c p) d -> p c d", p=P)
                nc.sync.dma_start(wk_t[:, 0 : CC // 2, :], wkr[:, 0 : CC // 2, :])
                xkr = kT_d.rearrange("(c p) s -> p c s", p=P)
                xvr = vT_d.rearrange("(c p) s -> p c s", p=P)
                wv_t = hwp.tile([P, CC, DSL], f16, tag="wv", name="wv_t")
                xv_t = hx.tile([P, CC, S], f16, tag="xv", name="xv_t")

                def xk_stripe(ns):
                    for hf in range(2):
                        nc.sync.dma_start(
                            xk_ts[hf][:, :, ns * QB : (ns + 1) * QB],
                            xkr[:, hf * 4 : hf * 4 + 4,
                                ns * QB : (ns + 1) * QB],
                        )

                xk_stripe(0)
                nc.sync.dma_start(wk_t[:, CC // 2 : CC, :],
                                  wkr[:, CC // 2 : CC, :])
                nc.sync.dma_start(bk_t[:], bk_d.rearrange("(o p) -> p o", p=P))
                xk_stripe(1)
                nc.sync.dma_start(wv_t[:], wvT_d.rearrange("(c p) d -> p c d", p=P))
                nc.sync.dma_start(bv16[:], bv_d.rearrange("(o d) -> o d", o=1))
                nc.sync.dma_start(xv_t[:, :, 0:QB], xvr[:, :, 0:QB])
                xk_stripe(2)
                nc.sync.dma_start(xv_t[:, :, QB : 2 * QB], xvr[:, :, QB : 2 * QB])
                xk_stripe(3)
                for st in range(2, 4):
                    nc.sync.dma_start(
                        xv_t[:, :, st * QB : (st + 1) * QB],
                        xvr[:, :, st * QB : (st + 1) * QB],
                    )
                nc.sync.dma_start(wq_t[:], wqT_d.rearrange("(c p) d -> p c d", p=P))
                nc.sync.dma_start(bq_t[:], bq_d.rearrange("(o p) -> p o", p=P))
                xqr = qT_d.rearrange("(c p) s -> p c s", p=P)
                for ns in range(4):
                    for hf in range(2):
                        nc.sync.dma_start(
                            xq_ts[hf][:, :, ns * QB : (ns + 1) * QB],
                            xqr[:, hf * 4 : hf * 4 + 4,
                                ns * QB : (ns + 1) * QB],
                        )
                nc.sync.dma_start(
                    woT_sb[:], woT_d.rearrange("(c p) e -> p c e", p=P)
                )

                # k proj ns 0-1 now; ns 2-3 become attention fillers
                for ns in range(2):
                    for mc in range(DSL // P):
                        emit_proj_chunk(KTZ, wk_t, bk_t, xk_ts, proj_ps, mc, ns,
                                        bias_on_act=True, split_halves=True)

                # V' direct: stationary = x s-chunk, moving = Wv^T; bias via
                # a K=1 ones-row matmul folded into the accumulation group.
                for sc in range(KC):
                    vps = v_ps.tile([P, HPG, D_K], f32, tag="vps", name=f"vps{sc}")
                    for cc in range(CC):
                        nc.tensor.matmul(
                            vps[:],
                            xv_t[:, cc, sc * P : (sc + 1) * P],
                            wv_t[:, cc, :],
                            start=(cc == 0),
                            stop=False,
                        )
                    nc.tensor.matmul(
                        vps[:],
                        ones16[0:1, 0:P],
                        bv16[0:1, :],
                        start=False,
                        stop=True,
                    )
                    nc.scalar.copy(vpc[:, sc, :, 0:D_K], vps[:])

                # q proj for qb0
                for mc in range(DSL // P):
                    emit_proj_chunk(QT, wq_t, bq_t, xq_ts, proj_ps, mc, 0,
                                    bias_on_act=True)

            # ---- attention ----
            with (
                tc.tile_pool(name="norm", bufs=2) as norm_pool,
                tc.tile_pool(name="y_out", bufs=2) as ypool,
                tc.tile_pool(name="sc_ps", bufs=2, space="PSUM") as sc_ps,
                tc.tile_pool(name="acc_ps", bufs=2, space="PSUM") as acc_ps,
                tc.tile_pool(name="rby_ps", bufs=2, space="PSUM") as rby_ps,
            ):
                def emit_norm_late(pend):
                    # per head: broadcast 1/sums (fp16 K=1 matmul from the
                    # per-qb reciprocal tile), scale the unnormalized
                    # [64, 512] head output into XT.
                    qb_, ctx, rec16 = pend
                    qs_ = slice(qb_ * QB, (qb_ + 1) * QB)
                    for h, xu in ctx:
                        hc, hp = divmod(h, 2)
                        rp = 32 * (h % 2)
                        rb_ps = rby_ps.tile(
                            [D_K, QB], f32, tag="rby", name=f"rb{h}"
                        )
                        nc.tensor.matmul(
                            rb_ps[:],
                            ones16[rp : rp + 1, 0:D_K],
                            rec16[rp : rp + 1, :],
                            start=True,
                            stop=True,
                        )
                        if hp == 0:
                            nc.vector.tensor_tensor(
                                XT[0:D_K, hc, qs_], xu[0:D_K, :], rb_ps[:],
                                mybir.AluOpType.mult,
                            )
                        else:
                            # partitions 64-127: normalize to a temp, then
                            # partition-shift with an SBUF->SBUF DMA.
                            tmp = norm_pool.tile([D_K, QB], f16, tag="xtmp")
                            nc.vector.tensor_tensor(
                                tmp[:], xu[0:D_K, :], rb_ps[:],
                                mybir.AluOpType.mult,
                            )
                            nc.sync.dma_start(XT[D_K:P, hc, qs_], tmp[:])

                def make_outproj(qb_, sc4, tail=False):
                    def emit():
                        sc = qb_ * 4 + sc4
                        ps2 = [
                            rby_ps.tile([P, QB], f32, tag="rby",
                                        name=f"yp{sc4}_{ec}")
                            for ec in range(2)
                        ]
                        for dc in range(DSL // P):
                            for ec in range(2):
                                nc.tensor.matmul(
                                    ps2[ec][:],
                                    XT[:, dc, sc * P : (sc + 1) * P],
                                    woT_sb[:, dc, ec * QB : (ec + 1) * QB],
                                    start=(dc == 0),
                                    stop=(dc == DSL // P - 1),
                                )
                        # at the tail ACT is idle; split the PSUM drains
                        if tail:
                            nc.scalar.copy(y_sb[:, sc4, 0:QB], ps2[0][:])
                        else:
                            nc.vector.tensor_copy(
                                y_sb[:, sc4, 0:QB], ps2[0][:]
                            )
                        nc.vector.tensor_copy(
                            y_sb[:, sc4, QB : 2 * QB], ps2[1][:]
                        )
                        nc.sync.dma_start(
                            y_r[:, sc : sc + 1, :], y_sb[:, sc4 : sc4 + 1, :]
                        )
                    return emit

                def make_qproj(nsq, mc):
                    def emit():
                        emit_proj_chunk(QT, wq_t, bq_t, xq_ts, rby_ps, mc, nsq)
                    return emit

                y_sb = ypool.tile([P, 4, HIDDEN], f32, tag="y", name="ysb",
                                  bufs=1)

                def make_kproj(ns, mc):
                    return lambda: emit_proj_chunk(
                        KTZ, wk_t, bk_t, xk_ts, rby_ps, mc, ns,
                        split_halves=True,
                    )

                pending_norm = None
                carry = []
                for qb in range(N_QB):
                    qs = slice(qb * QB, (qb + 1) * QB)
                    qA = (
                        [make_kproj(ns, mc)
                         for ns in range(2, 4) for mc in range(DSL // P)]
                        if qb == 0 else []
                    )
                    if qb + 1 < N_QB:
                        qA += [make_qproj(qb + 1, mc) for mc in range(DSL // P)]
                    qR = carry  # ready fillers carried across the qb boundary
                    qB = (
                        [make_outproj(qb - 1, sc4) for sc4 in range(4)]
                        if qb > 0 else []
                    )
                    for hpair in range(HPG // 2):
                        heads = (2 * hpair, 2 * hpair + 1)
                        accs = {}
                        for h in heads:
                            accs[h] = acc_ps.tile(
                                [P, QB], f32, tag="acc", name=f"acc{h}"
                            )
                        for g in range(NG + D):
                            if g == 2 and pending_norm is not None:
                                emit_norm_late(pending_norm)
                                pending_norm = None
                            if g < NG:
                                for hi, h in enumerate(heads):
                                    hc = h // 2
                                    sct = sc_ps.tile(
                                        [P, 2, QB], f32, tag="sc",
                                        name=f"sc{h}",
                                    )
                                    for j in range(2):
                                        kc = 2 * g + j
                                        nc.tensor.matmul(
                                            sct[:, j, :],
                                            KTZ[h % 2][:, hc,
                                                       kc * P : (kc + 1) * P],
                                            QT[:, hc, qs],
                                            start=True,
                                            stop=True,
                                        )
                                    nc.scalar.activation(
                                        expc[:, 2 * g : 2 * g + 2, hi, :],
                                        sct[:],
                                        mybir.ActivationFunctionType.Exp,
                                        scale=float(SCALE),
                                    )
                            # one filler per g keeps the PE busy through the
                            # exp latency; drain groups get extra budget to
                            # bridge the pair boundary where ACT lags
                            budget = 2 if (len(qA) > 6 or g >= NG) else 1
                            for _ in range(budget):
                                if qA:
                                    qA.pop(0)()
                                elif qR:
                                    qR.pop(0)()
                                elif qB and (hpair > 0 or g >= 4):
                                    qB.pop(0)()
                            if g >= D:
                                for hi, h in enumerate(heads):
                                    for j in range(2):
                                        kc = 2 * (g - D) + j
                                        nc.tensor.matmul(
                                            accs[h][:],
                                            vpc[:, kc, h, :],
                                            expc[:, kc, hi, :],
                                            start=(kc == 0),
                                            stop=(kc == KC - 1),
                                        )
                        # pair epilogue: move the unnormalized outputs off
                        # PSUM (frees acc slots), gather the two sums rows
                        # onto partitions {0, 32} with tiny SBUF DMAs, and
                        # take one reciprocal + fp16 cast for the pair --
                        # all off the PE critical path.
                        sums33 = norm_pool.tile([33, QB], f32, tag="sums",
                                                name=f"sums{hpair}")
                        nc.gpsimd.memset(sums33[:], 1.0)
                        ctx = []
                        for h in heads:
                            xu = norm_pool.tile([D_K + 1, QB], f32, tag="xu",
                                                name=f"xu{h}", bufs=4)
                            nc.vector.tensor_copy(xu[:], accs[h][0 : D_K + 1, :])
                            rp = 32 * (h % 2)
                            nc.sync.dma_start(
                                sums33[rp : rp + 1, :],
                                xu[D_K : D_K + 1, :],
                            )
                            ctx.append((h, xu))
                        rec32 = norm_pool.tile([33, QB], f32, tag="rec32",
                                               name=f"rc{hpair}")
                        nc.vector.reciprocal(rec32[:], sums33[:])
                        rec16 = norm_pool.tile([33, QB], f16, tag="rec16",
                                               name=f"rh{hpair}")
                        nc.vector.tensor_copy(rec16[:], rec32[:])
                        pending_norm = (qb, ctx, rec16)
                    # q/k proj fillers must land before the next qb needs
                    # them; outproj leftovers roll over as boundary fillers
                    while qA:
                        qA.pop(0)()
                    carry = qR + qB
                # tail: leftovers, last qb's normalization, last projection
                for f in carry:
                    f()
                emit_norm_late(pending_norm)
                for sc4 in range(4):
                    make_outproj(N_QB - 1, sc4)()

    nc.finalize()
    return nc


_NC_CACHE = None


def _get_nc():
    global _NC_CACHE
    if _NC_CACHE is None:
        _NC_CACHE = _build_nc()
    return _NC_CACHE


def make_in_maps(q, k, v, Wq, bq, Wk, bk, Wv, bv, Wo):
    """Host-side sharding: per-core input dicts (core = b * G + g)."""
    f16 = np.float16
    qT = [np.ascontiguousarray(q[b].T).astype(f16) for b in range(B)]
    kT = [np.ascontiguousarray(k[b].T).astype(f16) for b in range(B)]
    vT = [np.ascontiguousarray(v[b].T).astype(f16) for b in range(B)]
    in_maps = []
    for core in range(B * G):
        b, g = divmod(core, G)
        sl = slice(g * DSL, (g + 1) * DSL)
        in_maps.append(
            {
                "qT": qT[b],
                "kT": kT[b],
                "vT": vT[b],
                "wqT": np.ascontiguousarray(Wq[sl, :].T).astype(f16),
                "wkT": np.ascontiguousarray(Wk[sl, :].T).astype(f16),
                "wvT": np.ascontiguousarray(Wv[sl, :].T).astype(f16),
                "woT": np.ascontiguousarray(Wo[:, sl].T).astype(f16),
                "bq": np.ascontiguousarray(bq[sl], np.float32),
                "bk": np.ascontiguousarray(bk[sl], np.float32),
                "bv": np.ascontiguousarray(bv[sl]).astype(f16),
            }
        )
    return in_maps


def kernel(q, k, v, Wq, bq, Wk, bk, Wv, bv, Wo, bo):
    from concourse.bass_utils import run_bass_kernel_spmd

    q, k, v = (np.asarray(a, np.float32) for a in (q, k, v))
    Wq, Wk, Wv, Wo = (np.asarray(a, np.float32) for a in (Wq, Wk, Wv, Wo))
    bq, bk, bv, bo = (np.asarray(a, np.float32) for a in (bq, bk, bv, bo))

    nc = _get_nc()
    in_maps = make_in_maps(q, k, v, Wq, bq, Wk, bk, Wv, bv, Wo)
    res = run_bass_kernel_spmd(nc, in_maps, core_ids=list(range(B * G)))

    out = np.zeros((B, S, HIDDEN), np.float32)
    for b in range(B):
        acc = np.zeros((S, HIDDEN), np.float32)
        for g in range(G):
            acc += res.results[b * G + g]["y"]
        out[b] = acc + bo
    return out
